# revision 23
# baseline (speedup 1.0000x reference)
"""KingLoss Trainium2 kernel (raw Bass, explicit semaphores) — v5.

Masked cross-entropy loss over [N, 10] logits, data-parallel over 8
NeuronCores.  Each core reduces its shard of rows to tiny per-engine
partial sums on device; the host does the final (cheap) reduction.

Per-row math (epoch % 5 == 0 branch, the one the harness exercises):
    s_i    = sum_c exp(x_ic)
    lse_i  = ln(s_i)
    loss_i = lse_i - x_{i,t_i} + (t_i != K) * exp(x_{i,K} - lse_i)
    loss   = mean_i loss_i

Device produces three global sums: Sum lse (ACT Ln accumulator),
Sum x_t (PE column-sum of the one-hot product), and Sum (t!=K) p
(ACT Exp accumulator over a king-masked exponent).

Design (baseline was 214 us):
  * x sent bf16, host pre-transposed to class-major per-partition
    layout [P, C*R] (block c = R contiguous rows' class-c logits): all
    SBUF operands unit-stride, DMA 20KB/partition bursts.
  * one TT is_equal of a stride-0-broadcast t against a tiny [P, C]
    iota view builds the one-hot; one TT mult gathers x_t into prod.
  * dbm = -30*onehot_K + x_K (STT) makes exp(dbm - lse) the ALREADY
    king-masked p, so Sum (t!=K) p rides the ACT Exp accumulator free.
  * Row sums of exp: a1 (5R-wide) on DVE; a2/a3/s on Pool.  Pool
    (gpsimd) is ~4x slower per element, so it only gets R-sized adds,
    and the cross-engine chain a1->a2->a3->s->ln->db2->pexp is
    software-pipelined TWO tiles behind the DVE/ACT front (cmp, prod,
    dbm, a1, exp all read only tile-i inputs), which keeps every
    engine's queue busy and lets DMA slots recycle three tiles ahead.
  * The idle PE reduces prod: 20 ones-vector matmuls per tile
    accumulate column sums into one PSUM row across all tiles, drained
    once at the end (ACT copy -> SBUF -> DMA).

This container's walrus rejects custom-DVE ops, InstPool, TT divide,
non-arithmetic TT on Pool, and STT on Pool; ACT Reciprocal is banned in
bass; DVE 2-byte "2x" TT modes do not engage on this hardware.  The
measured rates driving the split: DVE TT 0.55-0.63 ns/elem, DVE STT
~1.1, ACT 0.86, Pool TT ~2.3-7 (erratic), PE matmul ~0.4us/512 cols.
"""

import os
import sys
from contextlib import ExitStack

import numpy as np

for _p in ("/opt/trn_rl_repo", "/root/.axon_site/_ro/trn_rl_repo"):
    if os.path.isdir(_p) and _p not in sys.path:
        sys.path.insert(0, _p)
        break

import ml_dtypes

import concourse.bass as bass
import concourse.mybir as mybir
from concourse.bass_utils import run_bass_kernel_spmd

P = 128          # SBUF partitions
C = 10           # classes
KING = 3
R = 1024         # rows per partition per tile
F = R * C        # elements per partition per x tile
N_CORES = 8
NBUF = 2         # et/prod/sf buffer rotation depth
XBUF = 2         # xt/tt buffer rotation depth
KBUF = 3         # dbmt buffer rotation depth (written i, read i+LAG)
BIG = 30.0       # exponent offset that zeroes king rows in p
LAG = 2          # software-pipeline distance of the lse/p tail

FP32 = mybir.dt.float32
BF16 = mybir.dt.bfloat16
AF = mybir.ActivationFunctionType
OP = mybir.AluOpType

_BUILT = {}
LAST = {}  # exec_time_ns etc. from the most recent run, for test harnesses


def _build_zero(T):
    """epoch % 5 == 0 branch.  T = tiles per core."""
    assert T > LAG
    nc = bass.Bass()
    x = nc.declare_dram_parameter("x", [T * P, F], BF16, isOutput=False)
    tg = nc.declare_dram_parameter("t", [T * P, R], BF16, isOutput=False)
    out_a = nc.declare_dram_parameter("pa", [P, 2 * T], FP32, isOutput=True)
    out_g = nc.declare_dram_parameter("pg", [1, 512], FP32, isOutput=True)

    R5 = 5 * R
    MM = 512                  # moving free dim per matmul
    NMM = F // MM             # matmuls per tile

    # ---- precomputed semaphore values (mirror emission order) --------
    # ACT : exp(i); tail(j=i-LAG): ln(j) [accum lse], pexp(j) [accum p]
    # DVE : ones memset; cmp(i), prod(i), dbm(i), a1(i); tail: db2(j)
    # Pool: a2(i), a3(i), s(i)
    # PE  : one inc per tile (after its 20 matmuls)
    CMP, PROD, DBM, A1, DB2 = {}, {}, {}, {}, {}
    EXPC, LN, PEXP = {}, {}, {}
    SC = {}
    PEC = {}
    n = 0
    for i in range(T):
        n += 1
        EXPC[i] = n
        if i >= LAG:
            n += 1
            LN[i - LAG] = n
        if i >= LAG + 1:
            n += 1
            PEXP[i - LAG - 1] = n
    for j in range(T - LAG, T):
        n += 1
        LN[j] = n
        n += 1
        PEXP[j - 1] = n
    n += 1
    PEXP[T - 1] = n
    n = 1
    for i in range(T):
        n += 1
        CMP[i] = n
        n += 1
        DBM[i] = n
        n += 1
        PROD[i] = n
        n += 1
        A1[i] = n
        if i >= LAG:
            n += 1
            DB2[i - LAG] = n
    for j in range(T - LAG, T):
        n += 1
        DB2[j] = n
    for i in range(T):
        SC[i] = 3 * i + 3
        PEC[i] = i + 1

    with ExitStack() as ctx:
        ec = ctx.enter_context
        xt = ec(nc.sbuf_tensor("xt", [P, XBUF * F], BF16))
        tt = ec(nc.sbuf_tensor("tt", [P, XBUF * R], BF16))
        et = ec(nc.sbuf_tensor("et", [P, NBUF * F], BF16))
        cmpb = ec(nc.sbuf_tensor("cmp", [P, NBUF * F], BF16))
        iot = ec(nc.sbuf_tensor("iot", [P, F], BF16))
        a1 = ec(nc.sbuf_tensor("a1", [P, NBUF * R5], BF16))
        a2 = ec(nc.sbuf_tensor("a2", [P, 2 * R], BF16))
        a3 = ec(nc.sbuf_tensor("a3", [P, R], BF16))
        sf = ec(nc.sbuf_tensor("sf", [P, NBUF * R], FP32))
        lse = ec(nc.sbuf_tensor("lse", [P, R], BF16))
        dbmt = ec(nc.sbuf_tensor("dbmt", [P, KBUF * R], BF16))
        db2b = ec(nc.sbuf_tensor("db2b", [P, NBUF * R], BF16))
        pb = ec(nc.sbuf_tensor("pb", [P, R], BF16))
        ones = ec(nc.sbuf_tensor("ones", [P, 1], BF16))
        sta = ec(nc.sbuf_tensor("sta", [P, 2 * T], FP32))
        gsb = ec(nc.sbuf_tensor("gsb", [P, MM], FP32))
        gs = ec(nc.psum_tensor("gs", [P, MM], FP32))
        dma_x0 = ec(nc.semaphore("dma_x0"))
        dma_x1 = ec(nc.semaphore("dma_x1"))
        dma_t0 = ec(nc.semaphore("dma_t0"))
        dma_t1 = ec(nc.semaphore("dma_t1"))
        act_sem = ec(nc.semaphore("act_sem"))
        dve_sem = ec(nc.semaphore("dve_sem"))
        pool_sem = ec(nc.semaphore("pool_sem"))
        pe_sem = ec(nc.semaphore("pe_sem"))
        dma_oa = ec(nc.semaphore("dma_oa"))
        dma_oc = ec(nc.semaphore("dma_oc"))
        block = ec(nc.Block())

        dma_x = [dma_x0, dma_x1]
        dma_t = [dma_t0, dma_t1]

        def xtile(b):
            return xt[:, b * F:(b + 1) * F]

        def ttile(b):
            return tt[:, b * R:(b + 1) * R]

        def etile(e):
            return et[:, e * F:(e + 1) * F]

        def ptile(e):
            return cmpb[:, e * F:(e + 1) * F]

        def ktile(k):
            return dbmt[:, k * R:(k + 1) * R]

        def a1tile(e):
            return a1[:, e * R5:(e + 1) * R5]

        def stile(e):
            return sf[:, e * R:(e + 1) * R]

        @block.sync
        def _(sync):
            for i in range(T):
                b = i % XBUF
                if i >= XBUF:
                    j = i - XBUF
                    # xt[b] readers: exp(j) ACT; dbm(j), prod(j) DVE.
                    # tt[b] reader: cmp(j) DVE.  PROD[j] covers both.
                    sync.wait_ge(act_sem, EXPC[j])
                    sync.wait_ge(dve_sem, PROD[j])
                    sync.wait_ge(dma_x[b], 16 * (i // XBUF))
                    sync.wait_ge(dma_t[b], 16 * (i // XBUF))
                sync.dma_start(
                    out=ttile(b), in_=tg[i * P:(i + 1) * P, :]
                ).then_inc(dma_t[b], 16)
                sync.dma_start(
                    out=xtile(b), in_=x[i * P:(i + 1) * P, :]
                ).then_inc(dma_x[b], 16)
            sync.wait_ge(act_sem, PEXP[T - 1])
            sync.dma_start(out=out_a[:, :], in_=sta[:, :]).then_inc(dma_oa, 16)
            # PSUM is not DMA-readable: ACT copies it to gsb first.
            sync.wait_ge(act_sem, PEXP[T - 1] + 1)
            sync.dma_start(out=out_g[:, :], in_=gsb[0:1, :]).then_inc(
                dma_oc, 16)
            sync.wait_ge(dma_oa, 16)
            sync.wait_ge(dma_oc, 16)

        # ---- ACT ------------------------------------------------------
        @block.scalar
        def _(scalar):
            def emit_ln(j):
                scalar.wait_ge(pool_sem, SC[j])           # s(j) ready
                if j >= 1:
                    scalar.wait_ge(dve_sem, DB2[j - 1])   # lse free
                scalar.activation(
                    lse[:, :], stile(j % NBUF), AF.Ln,
                    accum_out=sta[:, 2 * j:2 * j + 1],
                ).then_inc(act_sem, 1)

            def emit_pexp(j):
                scalar.wait_ge(dve_sem, DB2[j])           # db2(j) ready
                scalar.activation(
                    pb[:, :], db2b[:, (j % NBUF) * R:(j % NBUF + 1) * R],
                    AF.Exp,
                    accum_out=sta[:, 2 * j + 1:2 * j + 2],
                ).then_inc(act_sem, 1)

            for i in range(T):
                b = i % XBUF
                e = i % NBUF
                scalar.wait_ge(dma_x[b], 16 * (i // XBUF + 1))
                if i >= NBUF:
                    # et[e] reader: a1(i-NBUF) on DVE.
                    scalar.wait_ge(dve_sem, A1[i - NBUF])
                scalar.activation(etile(e), xtile(b), AF.Exp).then_inc(
                    act_sem, 1)
                if i >= LAG:
                    emit_ln(i - LAG)
                if i >= LAG + 1:
                    emit_pexp(i - LAG - 1)
            for j in range(T - LAG, T):
                emit_ln(j)
                emit_pexp(j - 1)
            emit_pexp(T - 1)
            # drain the PE accumulation to SBUF for the output DMA
            scalar.wait_ge(pe_sem, T)
            scalar.activation(gsb[0:1, :], gs[0:1, :], AF.Copy).then_inc(
                act_sem, 1)

        # ---- DVE ------------------------------------------------------
        @block.vector
        def _(vector):
            # preamble: iota blocks (c in block c) + PE's ones vector;
            # runs while tile-0 DMA streams in.
            for c in range(C):
                vector.memset(iot[:, c * R:(c + 1) * R], float(c))
            vector.memset(ones[:, :], 1.0).then_inc(dve_sem, 1)

            def emit_tail(j):
                vector.wait_ge(act_sem, LN[j])            # ln(j) done
                vector.tensor_tensor(
                    db2b[:, (j % NBUF) * R:(j % NBUF + 1) * R],
                    ktile(j % KBUF), lse[:, :], OP.subtract,
                ).then_inc(dve_sem, 1)

            for i in range(T):
                b = i % XBUF
                e = i % NBUF
                vector.wait_ge(dma_t[b], 16 * (i // XBUF + 1))
                if i >= NBUF:
                    # cmpb[e] doubles as prod: PE(i-NBUF) must be done.
                    vector.wait_ge(pe_sem, PEC[i - NBUF])
                tb3 = ttile(b).unsqueeze(1).to_broadcast([P, C, R])
                vector.tensor_tensor(
                    ptile(e).rearrange("p (c r) -> p c r", r=R),
                    tb3,
                    iot[:, :].rearrange("p (c r) -> p c r", r=R),
                    OP.is_equal,
                ).then_inc(dve_sem, 1)
                vector.wait_ge(dma_x[b], 16 * (i // XBUF + 1))
                vector.scalar_tensor_tensor(
                    ktile(i % KBUF), ptile(e)[:, KING * R:(KING + 1) * R],
                    -BIG, xtile(b)[:, KING * R:(KING + 1) * R],
                    OP.mult, OP.add,
                ).then_inc(dve_sem, 1)
                # gather product, in place over the one-hot
                vector.tensor_tensor(
                    ptile(e), ptile(e), xtile(b), OP.mult,
                ).then_inc(dve_sem, 1)
                vector.wait_ge(act_sem, EXPC[i])          # exp(i) done
                if i >= NBUF:
                    # a1[e] readers: a2/s of tile i-NBUF on Pool.
                    vector.wait_ge(pool_sem, SC[i - NBUF])
                vector.tensor_tensor(
                    a1tile(e), etile(e)[:, 0:R5], etile(e)[:, R5:2 * R5],
                    OP.add,
                ).then_inc(dve_sem, 1)
                if i >= LAG:
                    emit_tail(i - LAG)
            for j in range(T - LAG, T):
                emit_tail(j)

        # ---- Pool: small adds only -------------------------------------
        @block.gpsimd
        def _(gp):
            for i in range(T):
                e = i % NBUF
                gp.wait_ge(dve_sem, A1[i])
                a1t = a1tile(e)
                gp.tensor_tensor(
                    a2[:, :], a1t[:, 0:2 * R], a1t[:, 2 * R:4 * R], OP.add
                ).then_inc(pool_sem, 1)
                gp.tensor_tensor(
                    a3[:, :], a2[:, 0:R], a2[:, R:2 * R], OP.add
                ).then_inc(pool_sem, 1)
                if i >= NBUF:
                    # sf[e] free: ln(i-NBUF) must be done.
                    gp.wait_ge(act_sem, LN[i - NBUF])
                gp.tensor_tensor(
                    stile(e), a3[:, :], a1t[:, 4 * R:5 * R], OP.add
                ).then_inc(pool_sem, 1)

        # ---- PE: column sums of prod accumulate into one PSUM row ------
        @block.tensor
        def _(tensor):
            tensor.wait_ge(dve_sem, 1)                    # ones ready
            for i in range(T):
                e = i % NBUF
                tensor.wait_ge(dve_sem, PROD[i])
                for j in range(NMM):
                    ins = tensor.matmul(
                        gs[0:1, :],
                        ones[:, :],
                        ptile(e)[:, j * MM:(j + 1) * MM],
                        start=(i == 0 and j == 0),
                        stop=(i == T - 1 and j == NMM - 1),
                    )
                    if j == NMM - 1:
                        ins.then_inc(pe_sem, 1)

    return nc


def _build_nonzero(T):
    """epoch % 5 != 0 branch: loss_i = (t==K) * (lse_i - x_{i,K})."""
    nc = bass.Bass()
    x = nc.declare_dram_parameter("x", [T * P, F], BF16, isOutput=False)
    tg = nc.declare_dram_parameter("t", [T * P, R], BF16, isOutput=False)
    out_v = nc.declare_dram_parameter("pv", [P, 2 * T], FP32, isOutput=True)

    R5 = 5 * R

    with ExitStack() as ctx:
        ec = ctx.enter_context
        xt = ec(nc.sbuf_tensor("xt", [P, NBUF * F], BF16))
        et = ec(nc.sbuf_tensor("et", [P, NBUF * F], BF16))
        tt = ec(nc.sbuf_tensor("tt", [P, NBUF * R], BF16))
        a1 = ec(nc.sbuf_tensor("a1", [P, NBUF * R5], BF16))
        a2 = ec(nc.sbuf_tensor("a2", [P, 2 * R], BF16))
        a3 = ec(nc.sbuf_tensor("a3", [P, R], BF16))
        sf = ec(nc.sbuf_tensor("sf", [P, R], FP32))
        lse = ec(nc.sbuf_tensor("lse", [P, R], FP32))
        dm2 = ec(nc.sbuf_tensor("dm2", [P, R], FP32))
        stv = ec(nc.sbuf_tensor("stv", [P, 2 * T], FP32))
        dma_x0 = ec(nc.semaphore("dma_x0"))
        dma_x1 = ec(nc.semaphore("dma_x1"))
        dma_t0 = ec(nc.semaphore("dma_t0"))
        dma_t1 = ec(nc.semaphore("dma_t1"))
        act_sem = ec(nc.semaphore("act_sem"))
        dve_sem = ec(nc.semaphore("dve_sem"))
        pool_sem = ec(nc.semaphore("pool_sem"))
        dma_ob = ec(nc.semaphore("dma_ob"))
        block = ec(nc.Block())

        dma_x = [dma_x0, dma_x1]
        dma_t = [dma_t0, dma_t1]

        def xtile(b):
            return xt[:, b * F:(b + 1) * F]

        def etile(b):
            return et[:, b * F:(b + 1) * F]

        def ttile(b):
            return tt[:, b * R:(b + 1) * R]

        def a1tile(b):
            return a1[:, b * R5:(b + 1) * R5]

        # act: exp(2i+1), ln(2i+2)
        # dve: mlse(2i+1), mx(2i+2)
        # pool: a1(4i+1), a2(4i+2), a3(4i+3), s(4i+4)
        @block.sync
        def _(sync):
            for i in range(T):
                b = i % NBUF
                if i >= NBUF:
                    j = i - NBUF
                    sync.wait_ge(act_sem, 2 * j + 1)
                    sync.wait_ge(dve_sem, 2 * j + 2)  # mx(j) read xt[b]
                    sync.wait_ge(dma_x[b], 16 * (i // NBUF))
                    sync.wait_ge(dma_t[b], 16 * (i // NBUF))
                sync.dma_start(
                    out=xtile(b), in_=x[i * P:(i + 1) * P, :]
                ).then_inc(dma_x[b], 16)
                sync.dma_start(
                    out=ttile(b), in_=tg[i * P:(i + 1) * P, :]
                ).then_inc(dma_t[b], 16)
            sync.wait_ge(dve_sem, 2 * T)
            sync.dma_start(out=out_v[:, :], in_=stv[:, :]).then_inc(dma_ob, 16)
            sync.wait_ge(dma_ob, 16)

        @block.scalar
        def _(scalar):
            for i in range(T):
                b = i % NBUF
                scalar.wait_ge(dma_x[b], 16 * (i // NBUF + 1))
                if i >= NBUF:
                    scalar.wait_ge(pool_sem, 4 * (i - NBUF) + 1)
                scalar.activation(etile(b), xtile(b), AF.Exp).then_inc(
                    act_sem, 1)                                   # 2i+1
                scalar.wait_ge(pool_sem, 4 * i + 4)               # s(i) ready
                if i >= 1:
                    # lse single-buffered: mlse(i-1) must be done.
                    scalar.wait_ge(dve_sem, 2 * (i - 1) + 1)
                scalar.activation(lse[:, :], sf[:, :], AF.Ln).then_inc(
                    act_sem, 1)                                   # 2i+2

        @block.vector
        def _(vector):
            for i in range(T):
                b = i % NBUF
                vector.wait_ge(dma_t[b], 16 * (i // NBUF + 1))
                vector.wait_ge(act_sem, 2 * i + 2)                # ln(i) done
                vector.scalar_tensor_tensor(
                    dm2[:, :], ttile(b), float(KING), lse[:, :],
                    OP.is_equal, OP.mult,
                    accum_out=stv[:, 2 * i:2 * i + 1],
                ).then_inc(dve_sem, 1)                            # 2i+1
                vector.wait_ge(dma_x[b], 16 * (i // NBUF + 1))
                vector.scalar_tensor_tensor(
                    dm2[:, :], ttile(b), float(KING),
                    xtile(b)[:, KING * R:(KING + 1) * R],
                    OP.is_equal, OP.mult,
                    accum_out=stv[:, 2 * i + 1:2 * i + 2],
                ).then_inc(dve_sem, 1)                            # 2i+2

        @block.gpsimd
        def _(gp):
            for i in range(T):
                b = i % NBUF
                gp.wait_ge(act_sem, 2 * i + 1)                    # exp(i)
                if i >= NBUF:
                    gp.wait_ge(dve_sem, 2 * (i - NBUF) + 2)
                gp.tensor_tensor(
                    a1tile(b), etile(b)[:, 0:R5], etile(b)[:, R5:2 * R5],
                    OP.add,
                ).then_inc(pool_sem, 1)                           # 4i+1
                a1t = a1tile(b)
                gp.tensor_tensor(
                    a2[:, :], a1t[:, 0:2 * R], a1t[:, 2 * R:4 * R], OP.add
                ).then_inc(pool_sem, 1)                           # 4i+2
                gp.tensor_tensor(
                    a3[:, :], a2[:, 0:R], a2[:, R:2 * R], OP.add
                ).then_inc(pool_sem, 1)                           # 4i+3
                if i >= 1:
                    # sf single-buffered: ln(i-1) must be done.
                    gp.wait_ge(act_sem, 2 * (i - 1) + 2)
                gp.tensor_tensor(
                    sf[:, :], a3[:, :], a1t[:, 4 * R:5 * R], OP.add
                ).then_inc(pool_sem, 1)                           # 4i+4

    return nc


def kernel(output, target, epoch):
    x = np.asarray(output)
    tgt = np.asarray(target)
    epoch_zero = int(epoch) % 5 == 0
    N = x.shape[0]
    n_per = N // N_CORES
    assert N % N_CORES == 0 and n_per % (P * R) == 0
    T = n_per // (P * R)

    # class-major per-partition layout: [T*P, C*R] where block c of a
    # partition holds that partition's R rows' class-c logits.
    xr = x.reshape(N_CORES, T * P, R, C)
    xcm = np.ascontiguousarray(np.swapaxes(xr, 2, 3)).astype(
        ml_dtypes.bfloat16).reshape(N_CORES, T * P, F)
    tf = tgt.reshape(N_CORES, T * P, R).astype(ml_dtypes.bfloat16)

    key = (T, epoch_zero)
    if key not in _BUILT:
        _BUILT[key] = _build_zero(T) if epoch_zero else _build_nonzero(T)
    nc = _BUILT[key]

    in_maps = []
    for ci in range(N_CORES):
        in_maps.append({"x": xcm[ci], "t": tf[ci]})

    trace = bool(os.environ.get("KERNEL_TRACE"))
    res = run_bass_kernel_spmd(nc, in_maps, list(range(N_CORES)), trace=trace)
    LAST["exec_time_ns"] = res.exec_time_ns
    LAST["result"] = res

    if epoch_zero:
        s_lse = s_xt = s_p = 0.0
        for r in res.results:
            pa = r["pa"].astype(np.float64).reshape(P, T, 2)
            s_lse += float(pa[:, :, 0].sum())
            s_p += float(pa[:, :, 1].sum())
            s_xt += float(r["pg"].astype(np.float64).sum())
        loss = (s_lse - s_xt + s_p) / N
    else:
        kl = kx = 0.0
        for r in res.results:
            pv = r["pv"].astype(np.float64).reshape(P, T, 2)
            kl += float(pv[:, :, 0].sum())
            kx += float(pv[:, :, 1].sum())
        loss = (kl - kx) / N
    return np.float32(loss)
